# revision 34
# baseline (speedup 1.0000x reference)
"""Trainium2 Bass kernel for nn_EntityCell (scatter_memory).

Math (per batch row b, entity e):
    gates = sigmoid(sum_d(s * (h + k)))              [B, E]
    h_tilda = tanh(h @ U + k @ V + (s @ W)[:, None]) [B, E, D]
    updated = h + gates[:, :, None] * h_tilda
    out = updated / sqrt(max(sum_d(updated^2), 1e-12))

Sharding: pure data parallel over the batch dim across 8 NeuronCores.

Host-side layout prep (part of the sharding step in kernel()):
  - inputs cast to fp16 (rel err ~5e-4 vs the 2e-2 tolerance); output is
    stored fp16 on device and upcast to fp32 on the host.
  - prev/keys/enc are ALSO pre-transposed per 128-row chunk to d-major
    [chunk, D, E, rows] so the device needs no on-chip transposes at all
    (PE matmul contracts over the partition dim, which must be d).
    prev is additionally kept row-major for the update step: HBM cost of
    loading h twice is ~1.8us/chunk against >6us/chunk of engine time for
    on-chip transposition + PSUM evacuation.

Per-core dataflow (B_loc=1024 rows, 8 chunks of 128 rows):
  - DMA: 5 transfers/chunk (hT, kT into one stacked tile; sT; h row-major;
    store), every partition line >= 5KB contiguous.
  - DVE: one fused multiply t2T = [hT|kT] * broadcast(sT) feeds the gate
    reduction; per-e tensor_scalar g-mult (4x mode), one big h-add, the
    segmented sum(u^2) reduce, 1-step Newton rsqrt, per-e final scale.
  - PE: per-e ones-matmul gate reduction over d; per-e fp16 matmuls
    hT_e@U + kT_e@V + sT@W accumulated in fp32 PSUM.
  - Act: sigmoid, tanh evac from PSUM to fp16, and the u^2 square (keeps
    front-stage Act FIFO free of late-stage dependencies).
"""

import numpy as np
from contextlib import nullcontext as _nullctx

B, E, D = 8192, 20, 128
N_CORES = 8
B_LOC = B // N_CORES
CHUNK = 128
N_CHUNKS = B_LOC // CHUNK

_CACHE = {}


def _groups(total, size):
    out = []
    lo = 0
    while lo < total:
        out.append((lo, min(size, total - lo)))
        lo += min(size, total - lo)
    return out


def _build_nc(reps=1, loop_n=None, ablate=None, gate_mode='fused',
              sq_eng='act', upd_mode='ts', eg=4, newton_iters=1,
              io_bufs=4, tr_bufs=3, bf_bufs=3, psm_bufs=4, psg_bufs=1,
              scale_eng='dve', add_eng='dve', h_add='row', hg=10,
              psh_bufs=2, epi='de'):
    import concourse.tile as tile
    from concourse import bacc, mybir
    from contextlib import ExitStack

    fp32 = mybir.dt.float32
    fp16 = mybir.dt.float16
    int32 = mybir.dt.int32
    AF = mybir.ActivationFunctionType
    OP = mybir.AluOpType

    nc = bacc.Bacc("TRN2", target_bir_lowering=False, debug=False)
    prevT_d = nc.declare_dram_parameter("prevT", [N_CHUNKS, D, E, CHUNK], fp16, isOutput=False)
    keysT_d = nc.declare_dram_parameter("keysT", [N_CHUNKS, D, E, CHUNK], fp16, isOutput=False)
    encT_d = nc.declare_dram_parameter("encT", [N_CHUNKS, D, CHUNK], fp16, isOutput=False)
    if epi == 'de':
        # row-major h and the output both live in [rows, D, E] layout so
        # per-(row,e) scalars broadcast along the MIDDLE dim (2x DVE mode)
        prev_d = nc.declare_dram_parameter("prevR", [B_LOC, D, E], fp16, isOutput=False)
        out_d = nc.declare_dram_parameter("out", [B_LOC, D, E], fp16, isOutput=True)
    else:
        prev_d = nc.declare_dram_parameter("prev", [B_LOC, E, D], fp16, isOutput=False)
        out_d = nc.declare_dram_parameter("out", [B_LOC, E, D], fp16, isOutput=True)
    u_d = nc.declare_dram_parameter("U", [D, D], fp16, isOutput=False)
    v_d = nc.declare_dram_parameter("V", [D, D], fp16, isOutput=False)
    w_d = nc.declare_dram_parameter("W", [D, D], fp16, isOutput=False)

    prev_v = prev_d[:].rearrange("(n p) x y -> n p (x y)", p=CHUNK)
    out_v = out_d[:].rearrange("(n p) x y -> n p (x y)", p=CHUNK)

    G_MAIN = _groups(E, eg)

    with ExitStack() as ctx:
        tc = ctx.enter_context(tile.TileContext(nc))
        const_pool = ctx.enter_context(tc.tile_pool(name="const", bufs=1))
        io_pool = ctx.enter_context(tc.tile_pool(name="io", bufs=io_bufs))
        tr_pool = ctx.enter_context(tc.tile_pool(name="tr", bufs=tr_bufs))
        bf_pool = ctx.enter_context(tc.tile_pool(name="bf", bufs=bf_bufs))
        sm_pool = ctx.enter_context(tc.tile_pool(name="sm", bufs=6))
        if h_add == 'pe' and psm_bufs > 3:
            psm_bufs = 3
        psm_pool = ctx.enter_context(tc.tile_pool(name="psm", bufs=psm_bufs, space="PSUM"))
        psg_pool = ctx.enter_context(tc.tile_pool(name="psg", bufs=psg_bufs, space="PSUM"))
        psh_pool = (
            ctx.enter_context(tc.tile_pool(name="psh", bufs=psh_bufs, space="PSUM"))
            if h_add == 'pe' else None
        )
        G_H = _groups(E, hg)

        # ---- constants ----
        u16c = const_pool.tile([D, D], fp16)
        v16c = const_pool.tile([D, D], fp16)
        w16c = const_pool.tile([D, D], fp16)
        nc.sync.dma_start(u16c[:], u_d[:])
        nc.sync.dma_start(v16c[:], v_d[:])
        nc.sync.dma_start(w16c[:], w_d[:])
        ones16 = const_pool.tile([D, 1], fp16)
        nc.gpsimd.memset(ones16[:], 1.0)
        magic = const_pool.tile([CHUNK, E], int32)
        nc.gpsimd.memset(magic[:], 0x5F3759DF)
        if h_add == 'pe':
            from concourse.masks import make_identity
            ident16 = const_pool.tile([D, D], fp16)
            make_identity(nc, ident16[:])

        loop_cm = (
            tc.For_i(0, loop_n, 1, hint_engines=tuple(mybir.ALL_ENGINES))
            if loop_n is not None
            else _nullctx()
        )
        with loop_cm:
         for cp in range(N_CHUNKS * reps):
            n = cp % N_CHUNKS
            # ---- loads (hT/kT pre-transposed on host) ----
            hkT = tr_pool.tile([D, 2, E, CHUNK], fp16, name="hkT")
            nc.sync.dma_start(
                hkT[:, 0].rearrange("p e c -> p (e c)"),
                prevT_d[n].rearrange("p e c -> p (e c)"),
            )
            nc.sync.dma_start(
                hkT[:, 1].rearrange("p e c -> p (e c)"),
                keysT_d[n].rearrange("p e c -> p (e c)"),
            )
            sT = tr_pool.tile([D, CHUNK], fp16, name="sT")
            nc.sync.dma_start(sT[:], encT_d[n])
            if h_add == 'pe':
                h16 = None
            else:
                hshape = [CHUNK, D, E] if epi == 'de' else [CHUNK, E, D]
                h16 = io_pool.tile(hshape, fp16, name="h16")
                nc.sync.dma_start(h16[:].rearrange("p x y -> p (x y)"), prev_v[n])

            if ablate == 'dma':
                if h16 is None:
                    nc.sync.dma_start(
                        out=out_v[n],
                        in_=hkT[:, 0].rearrange("p e d -> p (e d)"),
                    )
                else:
                    nc.sync.dma_start(
                        out=out_v[n], in_=h16[:].rearrange("p x y -> p (x y)")
                    )
                continue

            # ---- gates ----
            g32 = sm_pool.tile([CHUNK, E], fp32, name="g32")
            t2T = tr_pool.tile([D, 2, E, CHUNK], fp16, name="t2T")
            sTb = sT[:].unsqueeze(1).broadcast_to([D, 2 * E, CHUNK])
            nc.vector.tensor_tensor(
                t2T[:].rearrange("d a e c -> d (a e) c"),
                hkT[:].rearrange("d a e c -> d (a e) c"),
                sTb, OP.mult,
            )
            gps = psg_pool.tile([CHUNK, E], fp32, name="gps")
            for e in range(E):
                nc.tensor.matmul(
                    gps[:, e:e + 1], t2T[:, 0, e], ones16[:],
                    start=True, stop=False,
                )
                nc.tensor.matmul(
                    gps[:, e:e + 1], t2T[:, 1, e], ones16[:],
                    start=False, stop=True,
                )
            nc.scalar.activation(g32[:], gps[:], AF.Sigmoid)

            if ablate == 'gates':
                nc.sync.dma_start(
                    out=out_v[n],
                    in_=t2T[:, 0].rearrange("p e d -> p (e d)"),
                )
                continue

            # ---- main matmuls + tanh ----
            ht_shape = [CHUNK, D, E] if epi == 'de' else [CHUNK, E, D]
            ht16 = bf_pool.tile(ht_shape, fp16, name="ht16")
            for lo, sz in G_MAIN:
                ps = psm_pool.tile([CHUNK, eg, D], fp32, name="ps")
                for j in range(sz):
                    e = lo + j
                    nc.tensor.matmul(
                        ps[:, j], hkT[:, 0, e], u16c[:], start=True, stop=False
                    )
                    nc.tensor.matmul(
                        ps[:, j], hkT[:, 1, e], v16c[:], start=False, stop=False
                    )
                    nc.tensor.matmul(
                        ps[:, j], sT[:], w16c[:], start=False, stop=True
                    )
                if epi == 'de':
                    nc.scalar.activation(
                        ht16[:, :, lo:lo + sz].rearrange("p d e -> p e d"),
                        ps[:, :sz], AF.Tanh,
                    )
                else:
                    nc.scalar.activation(
                        ht16[:, lo:lo + sz], ps[:, :sz], AF.Tanh
                    )

            if ablate == 'compute':
                nc.sync.dma_start(
                    out=out_v[n], in_=ht16[:].rearrange("p x y -> p (x y)")
                )
                continue

            # ---- update u = g*t + h ----
            u16 = bf_pool.tile(ht_shape, fp16, name="u16")
            if epi == 'de':
                g16 = sm_pool.tile([CHUNK, E], fp16, name="g16")
                nc.scalar.copy(g16[:], g32[:])
                gb = g16[:].unsqueeze(1).broadcast_to([CHUNK, D, E])
                nc.vector.tensor_tensor(u16[:], ht16[:], gb, OP.mult)
                nc.vector.tensor_tensor(u16[:], u16[:], h16[:], OP.add)
            elif h_add == 'pe':
                # reconstruct row-major h into PSUM via PE transpose-back
                hps = []
                for lo, sz in G_H:
                    ph = psh_pool.tile([CHUNK, hg, D], fp16, name="ph")
                    for j in range(sz):
                        nc.tensor.transpose(
                            ph[:, j], hkT[:, 0, lo + j], ident16[:]
                        )
                    hps.append((ph, lo, sz))
                for e in range(E):
                    nc.vector.tensor_scalar(
                        u16[:, e], ht16[:, e], g32[:, e:e + 1], None,
                        op0=OP.mult,
                    )
                for ph, lo, sz in hps:
                    nc.vector.tensor_tensor(
                        u16[:, lo:lo + sz], u16[:, lo:lo + sz], ph[:, :sz],
                        OP.add,
                    )
            elif upd_mode == 'stt':
                for e in range(E):
                    nc.vector.scalar_tensor_tensor(
                        u16[:, e], ht16[:, e], g32[:, e:e + 1], h16[:, e],
                        OP.mult, OP.add,
                    )
            else:  # 'ts': per-e 4x-mode TS then one big add
                if upd_mode == 'ts16':
                    g16s = sm_pool.tile([CHUNK, E], fp16, name="g16s")
                    nc.scalar.copy(g16s[:], g32[:])
                    gsrc = g16s
                else:
                    gsrc = g32
                for e in range(E):
                    nc.vector.tensor_scalar(
                        u16[:, e], ht16[:, e], gsrc[:, e:e + 1], None,
                        op0=OP.mult,
                    )
                if add_eng == 'gps':
                    nc.gpsimd.tensor_tensor(u16[:], u16[:], h16[:], OP.add)
                else:
                    nc.vector.tensor_tensor(u16[:], u16[:], h16[:], OP.add)

            if ablate == 'update':
                nc.sync.dma_start(
                    out=out_v[n], in_=u16[:].rearrange("p x y -> p (x y)")
                )
                continue

            # ---- sum(u^2) ----
            u2 = bf_pool.tile(ht_shape, fp16, name="u2")
            if sq_eng == 'act':
                nc.scalar.activation(u2[:], u16[:], AF.Square)
            elif sq_eng == 'gps':
                nc.gpsimd.tensor_tensor(u2[:], u16[:], u16[:], OP.mult)
            else:
                nc.vector.tensor_tensor(u2[:], u16[:], u16[:], OP.mult)
            ss = sm_pool.tile([CHUNK, E], fp32, name="ss")
            u2v = (
                u2[:].rearrange("p d e -> p e d") if epi == 'de' else u2[:]
            )
            nc.vector.tensor_reduce(
                ss[:], u2v, axis=mybir.AxisListType.X, op=OP.add
            )

            # ---- r = rsqrt(ss): bit-trick seed + Newton ----
            ti = sm_pool.tile([CHUNK, E], int32, name="ti")
            nc.vector.tensor_scalar(
                ti[:], ss[:].bitcast(int32), 1, None,
                op0=OP.logical_shift_right,
            )
            yi = sm_pool.tile([CHUNK, E], int32, name="yi")
            nc.vector.tensor_tensor(yi[:], magic[:], ti[:], OP.subtract)
            y = yi[:].bitcast(fp32)
            for _ in range(newton_iters):
                y2 = sm_pool.tile([CHUNK, E], fp32, name="y2")
                nc.vector.tensor_tensor(y2[:], y, y, OP.mult)
                tt = sm_pool.tile([CHUNK, E], fp32, name="tt")
                nc.vector.tensor_tensor(tt[:], ss[:], y2[:], OP.mult)
                ww = sm_pool.tile([CHUNK, E], fp32, name="ww")
                nc.vector.tensor_scalar(
                    ww[:], tt[:], -0.5, 1.5, op0=OP.mult, op1=OP.add
                )
                yn = sm_pool.tile([CHUNK, E], fp32, name="yn")
                nc.vector.tensor_tensor(yn[:], y, ww[:], OP.mult)
                y = yn[:]

            if ablate == 'norm':
                nc.sync.dma_start(
                    out=out_v[n], in_=u16[:].rearrange("p x y -> p (x y)")
                )
                continue

            # ---- scale and store fp16 ----
            o16 = bf_pool.tile(ht_shape, fp16, name="o16")
            if epi == 'de':
                y16 = sm_pool.tile([CHUNK, E], fp16, name="y16")
                nc.vector.tensor_copy(y16[:], y)
                yb = y16[:].unsqueeze(1).broadcast_to([CHUNK, D, E])
                nc.vector.tensor_tensor(o16[:], u16[:], yb, OP.mult)
            elif scale_eng == 'bcast':
                y16 = sm_pool.tile([CHUNK, E], fp16, name="y16")
                nc.vector.tensor_copy(y16[:], y)
                yb = y16[:].unsqueeze(2).broadcast_to([CHUNK, E, D])
                nc.vector.tensor_tensor(o16[:], u16[:], yb, OP.mult)
            else:
                if scale_eng == 'ts16':
                    y16 = sm_pool.tile([CHUNK, E], fp16, name="y16")
                    nc.vector.tensor_copy(y16[:], y)
                    ysrc = y16[:]
                else:
                    ysrc = y
                for e in range(E):
                    if scale_eng == 'act':
                        nc.scalar.mul(o16[:, e], u16[:, e], ysrc[:, e:e + 1])
                    else:
                        nc.vector.tensor_scalar(
                            o16[:, e], u16[:, e], ysrc[:, e:e + 1], None, op0=OP.mult
                        )
            nc.sync.dma_start(
                out=out_v[n], in_=o16[:].rearrange("p x y -> p (x y)")
            )

    nc.compile()
    return nc


def _get_nc():
    if "nc" not in _CACHE:
        _CACHE["nc"] = _build_nc()
    return _CACHE["nc"]


def make_in_maps(encoded_sents, prev_states, keys, U, V, W):
    enc = np.asarray(encoded_sents, dtype=np.float16)
    prev = np.asarray(prev_states, dtype=np.float16)
    kys = np.asarray(keys, dtype=np.float16)
    U = np.ascontiguousarray(np.asarray(U, dtype=np.float16))
    V = np.ascontiguousarray(np.asarray(V, dtype=np.float16))
    W = np.ascontiguousarray(np.asarray(W, dtype=np.float16))
    # per-chunk d-major transposes: [B_LOC, E, D] -> [N_CHUNKS, D, E, CHUNK]
    prevT = np.ascontiguousarray(
        prev.reshape(N_CORES, N_CHUNKS, CHUNK, E, D).transpose(0, 1, 4, 3, 2)
    )
    keysT = np.ascontiguousarray(
        kys.reshape(N_CORES, N_CHUNKS, CHUNK, E, D).transpose(0, 1, 4, 3, 2)
    )
    encT = np.ascontiguousarray(
        enc.reshape(N_CORES, N_CHUNKS, CHUNK, D).transpose(0, 1, 3, 2)
    )
    prev_row = np.ascontiguousarray(prev)
    # [B_LOC, E, D] -> [B_LOC, D, E] for the d-major epilogue layout
    prevR = np.ascontiguousarray(prev.transpose(0, 2, 1))
    in_maps = []
    for i in range(N_CORES):
        lo, hi = i * B_LOC, (i + 1) * B_LOC
        in_maps.append(
            {
                "prevT": prevT[i],
                "keysT": keysT[i],
                "encT": encT[i],
                "prev": prev_row[lo:hi],
                "prevR": prevR[lo:hi],
                "U": U,
                "V": V,
                "W": W,
            }
        )
    return in_maps


def kernel(encoded_sents, prev_states, keys, U, V, W):
    import sys

    if "/opt/trn_rl_repo" not in sys.path:
        sys.path.insert(0, "/opt/trn_rl_repo")
    from concourse.bass_utils import run_bass_kernel_spmd

    nc = _get_nc()
    in_maps = make_in_maps(encoded_sents, prev_states, keys, U, V, W)
    res = run_bass_kernel_spmd(nc, in_maps, list(range(N_CORES)))
    out = np.concatenate([res.results[i]["out"] for i in range(N_CORES)], axis=0)
    # default build uses the d-major epilogue: device output is [B, D, E]
    if out.shape[1] == D:
        out = out.transpose(0, 2, 1)
    return out.astype(np.float32)


# revision 36
# speedup vs baseline: 1.6129x; 1.6129x over previous
"""Trainium2 Bass kernel for nn_EntityCell (scatter_memory).

Math (per batch row b, entity e):
    gates = sigmoid(sum_d(s * (h + k)))              [B, E]
    h_tilda = tanh(h @ U + k @ V + (s @ W)[:, None]) [B, E, D]
    updated = h + gates[:, :, None] * h_tilda
    out = updated / sqrt(max(sum_d(updated^2), 1e-12))

Sharding: pure data parallel over the batch dim across 8 NeuronCores.

Host-side layout prep (part of the sharding step in kernel()):
  - inputs cast to fp16 (rel err ~5e-4 vs the 2e-2 tolerance); output is
    stored fp16 on device and upcast to fp32 on the host.
  - prev/keys/enc are ALSO pre-transposed per 128-row chunk to d-major
    [chunk, D, E, rows] so the device needs no on-chip transposes at all
    (PE matmul contracts over the partition dim, which must be d).
    prev is additionally kept row-major for the update step: HBM cost of
    loading h twice is ~1.8us/chunk against >6us/chunk of engine time for
    on-chip transposition + PSUM evacuation.

Per-core dataflow (B_loc=1024 rows, 8 chunks of 128 rows):
  - DMA: 5 transfers/chunk (hT, kT into one stacked tile; sT; h row-major;
    store), every partition line >= 5KB contiguous.
  - DVE: one fused multiply t2T = [hT|kT] * broadcast(sT) feeds the gate
    reduction; per-e tensor_scalar g-mult (4x mode), one big h-add, the
    segmented sum(u^2) reduce, 1-step Newton rsqrt, per-e final scale.
  - PE: per-e ones-matmul gate reduction over d; per-e fp16 matmuls
    hT_e@U + kT_e@V + sT@W accumulated in fp32 PSUM.
  - Act: sigmoid, tanh evac from PSUM to fp16, and the u^2 square (keeps
    front-stage Act FIFO free of late-stage dependencies).
"""

import numpy as np
from contextlib import nullcontext as _nullctx

B, E, D = 8192, 20, 128
N_CORES = 8
B_LOC = B // N_CORES
CHUNK = 128
N_CHUNKS = B_LOC // CHUNK

_CACHE = {}


def _groups(total, size):
    out = []
    lo = 0
    while lo < total:
        out.append((lo, min(size, total - lo)))
        lo += min(size, total - lo)
    return out


def _build_nc(reps=1, loop_n=None, ablate=None, gate_mode='fused',
              sq_eng='act', upd_mode='ts', eg=4, newton_iters=1,
              io_bufs=4, tr_bufs=3, bf_bufs=3, psm_bufs=4, psg_bufs=2,
              scale_eng='dve', add_eng='dve', h_add='row', hg=10,
              psh_bufs=2, epi='ed'):
    import concourse.tile as tile
    from concourse import bacc, mybir
    from contextlib import ExitStack

    fp32 = mybir.dt.float32
    fp16 = mybir.dt.float16
    int32 = mybir.dt.int32
    AF = mybir.ActivationFunctionType
    OP = mybir.AluOpType

    nc = bacc.Bacc("TRN2", target_bir_lowering=False, debug=False)
    prevT_d = nc.declare_dram_parameter("prevT", [N_CHUNKS, D, E, CHUNK], fp16, isOutput=False)
    keysT_d = nc.declare_dram_parameter("keysT", [N_CHUNKS, D, E, CHUNK], fp16, isOutput=False)
    encT_d = nc.declare_dram_parameter("encT", [N_CHUNKS, D, CHUNK], fp16, isOutput=False)
    if epi == 'de':
        # row-major h and the output both live in [rows, D, E] layout so
        # per-(row,e) scalars broadcast along the MIDDLE dim (2x DVE mode)
        prev_d = nc.declare_dram_parameter("prevR", [B_LOC, D, E], fp16, isOutput=False)
        out_d = nc.declare_dram_parameter("out", [B_LOC, D, E], fp16, isOutput=True)
    else:
        prev_d = nc.declare_dram_parameter("prev", [B_LOC, E, D], fp16, isOutput=False)
        out_d = nc.declare_dram_parameter("out", [B_LOC, E, D], fp16, isOutput=True)
    u_d = nc.declare_dram_parameter("U", [D, D], fp16, isOutput=False)
    v_d = nc.declare_dram_parameter("V", [D, D], fp16, isOutput=False)
    w_d = nc.declare_dram_parameter("W", [D, D], fp16, isOutput=False)

    prev_v = prev_d[:].rearrange("(n p) x y -> n p (x y)", p=CHUNK)
    out_v = out_d[:].rearrange("(n p) x y -> n p (x y)", p=CHUNK)

    G_MAIN = _groups(E, eg)

    with ExitStack() as ctx:
        tc = ctx.enter_context(tile.TileContext(nc))
        const_pool = ctx.enter_context(tc.tile_pool(name="const", bufs=1))
        io_pool = ctx.enter_context(tc.tile_pool(name="io", bufs=io_bufs))
        tr_pool = ctx.enter_context(tc.tile_pool(name="tr", bufs=tr_bufs))
        bf_pool = ctx.enter_context(tc.tile_pool(name="bf", bufs=bf_bufs))
        sm_pool = ctx.enter_context(tc.tile_pool(name="sm", bufs=6))
        if h_add == 'pe' and psm_bufs > 3:
            psm_bufs = 3
        psm_pool = ctx.enter_context(tc.tile_pool(name="psm", bufs=psm_bufs, space="PSUM"))
        psg_pool = ctx.enter_context(tc.tile_pool(name="psg", bufs=psg_bufs, space="PSUM"))
        psh_pool = (
            ctx.enter_context(tc.tile_pool(name="psh", bufs=psh_bufs, space="PSUM"))
            if h_add == 'pe' else None
        )
        G_H = _groups(E, hg)

        # ---- constants ----
        u16c = const_pool.tile([D, D], fp16)
        v16c = const_pool.tile([D, D], fp16)
        w16c = const_pool.tile([D, D], fp16)
        nc.sync.dma_start(u16c[:], u_d[:])
        nc.sync.dma_start(v16c[:], v_d[:])
        nc.sync.dma_start(w16c[:], w_d[:])
        ones16 = const_pool.tile([D, 1], fp16)
        nc.gpsimd.memset(ones16[:], 1.0)
        magic = const_pool.tile([CHUNK, E], int32)
        nc.gpsimd.memset(magic[:], 0x5F3759DF)
        if h_add == 'pe':
            from concourse.masks import make_identity
            ident16 = const_pool.tile([D, D], fp16)
            make_identity(nc, ident16[:])

        loop_cm = (
            tc.For_i(0, loop_n, 1, hint_engines=tuple(mybir.ALL_ENGINES))
            if loop_n is not None
            else _nullctx()
        )
        with loop_cm:
         for cp in range(N_CHUNKS * reps):
            n = cp % N_CHUNKS
            # ---- loads (hT/kT pre-transposed on host) ----
            hkT = tr_pool.tile([D, 2, E, CHUNK], fp16, name="hkT")
            nc.sync.dma_start(
                hkT[:, 0].rearrange("p e c -> p (e c)"),
                prevT_d[n].rearrange("p e c -> p (e c)"),
            )
            nc.sync.dma_start(
                hkT[:, 1].rearrange("p e c -> p (e c)"),
                keysT_d[n].rearrange("p e c -> p (e c)"),
            )
            sT = tr_pool.tile([D, CHUNK], fp16, name="sT")
            nc.sync.dma_start(sT[:], encT_d[n])
            if h_add == 'pe':
                h16 = None
            else:
                hshape = [CHUNK, D, E] if epi == 'de' else [CHUNK, E, D]
                h16 = io_pool.tile(hshape, fp16, name="h16")
                nc.sync.dma_start(h16[:].rearrange("p x y -> p (x y)"), prev_v[n])

            if ablate == 'dma':
                if h16 is None:
                    nc.sync.dma_start(
                        out=out_v[n],
                        in_=hkT[:, 0].rearrange("p e d -> p (e d)"),
                    )
                else:
                    nc.sync.dma_start(
                        out=out_v[n], in_=h16[:].rearrange("p x y -> p (x y)")
                    )
                continue

            # ---- gates ----
            g32 = sm_pool.tile([CHUNK, E], fp32, name="g32")
            t2T = tr_pool.tile([D, 2, E, CHUNK], fp16, name="t2T")
            sTb = sT[:].unsqueeze(1).broadcast_to([D, 2 * E, CHUNK])
            nc.vector.tensor_tensor(
                t2T[:].rearrange("d a e c -> d (a e) c"),
                hkT[:].rearrange("d a e c -> d (a e) c"),
                sTb, OP.mult,
            )
            gps = psg_pool.tile([CHUNK, E], fp32, name="gps")
            for e in range(E):
                nc.tensor.matmul(
                    gps[:, e:e + 1], t2T[:, 0, e], ones16[:],
                    start=True, stop=False,
                )
                nc.tensor.matmul(
                    gps[:, e:e + 1], t2T[:, 1, e], ones16[:],
                    start=False, stop=True,
                )
            nc.scalar.activation(g32[:], gps[:], AF.Sigmoid)

            if ablate == 'gates':
                nc.sync.dma_start(
                    out=out_v[n],
                    in_=t2T[:, 0].rearrange("p e d -> p (e d)"),
                )
                continue

            # ---- main matmuls + tanh ----
            ht_shape = [CHUNK, D, E] if epi == 'de' else [CHUNK, E, D]
            ht16 = bf_pool.tile(ht_shape, fp16, name="ht16")
            for lo, sz in G_MAIN:
                ps = psm_pool.tile([CHUNK, eg, D], fp32, name="ps")
                for j in range(sz):
                    e = lo + j
                    nc.tensor.matmul(
                        ps[:, j], hkT[:, 0, e], u16c[:], start=True, stop=False
                    )
                    nc.tensor.matmul(
                        ps[:, j], hkT[:, 1, e], v16c[:], start=False, stop=False
                    )
                    nc.tensor.matmul(
                        ps[:, j], sT[:], w16c[:], start=False, stop=True
                    )
                if epi == 'de':
                    nc.scalar.activation(
                        ht16[:, :, lo:lo + sz].rearrange("p d e -> p e d"),
                        ps[:, :sz], AF.Tanh,
                    )
                else:
                    nc.scalar.activation(
                        ht16[:, lo:lo + sz], ps[:, :sz], AF.Tanh
                    )

            if ablate == 'compute':
                nc.sync.dma_start(
                    out=out_v[n], in_=ht16[:].rearrange("p x y -> p (x y)")
                )
                continue

            # ---- update u = g*t + h ----
            u16 = bf_pool.tile(ht_shape, fp16, name="u16")
            if epi == 'de':
                g16 = sm_pool.tile([CHUNK, E], fp16, name="g16")
                nc.scalar.copy(g16[:], g32[:])
                gb = g16[:].unsqueeze(1).broadcast_to([CHUNK, D, E])
                nc.vector.tensor_tensor(u16[:], ht16[:], gb, OP.mult)
                nc.vector.tensor_tensor(u16[:], u16[:], h16[:], OP.add)
            elif h_add == 'pe':
                # reconstruct row-major h into PSUM via PE transpose-back
                hps = []
                for lo, sz in G_H:
                    ph = psh_pool.tile([CHUNK, hg, D], fp16, name="ph")
                    for j in range(sz):
                        nc.tensor.transpose(
                            ph[:, j], hkT[:, 0, lo + j], ident16[:]
                        )
                    hps.append((ph, lo, sz))
                for e in range(E):
                    nc.vector.tensor_scalar(
                        u16[:, e], ht16[:, e], g32[:, e:e + 1], None,
                        op0=OP.mult,
                    )
                for ph, lo, sz in hps:
                    nc.vector.tensor_tensor(
                        u16[:, lo:lo + sz], u16[:, lo:lo + sz], ph[:, :sz],
                        OP.add,
                    )
            elif upd_mode == 'stt':
                for e in range(E):
                    nc.vector.scalar_tensor_tensor(
                        u16[:, e], ht16[:, e], g32[:, e:e + 1], h16[:, e],
                        OP.mult, OP.add,
                    )
            else:  # 'ts': per-e 4x-mode TS then one big add
                if upd_mode == 'ts16':
                    g16s = sm_pool.tile([CHUNK, E], fp16, name="g16s")
                    nc.scalar.copy(g16s[:], g32[:])
                    gsrc = g16s
                else:
                    gsrc = g32
                for e in range(E):
                    nc.vector.tensor_scalar(
                        u16[:, e], ht16[:, e], gsrc[:, e:e + 1], None,
                        op0=OP.mult,
                    )
                if add_eng == 'gps':
                    nc.gpsimd.tensor_tensor(u16[:], u16[:], h16[:], OP.add)
                else:
                    nc.vector.tensor_tensor(u16[:], u16[:], h16[:], OP.add)

            if ablate == 'update':
                nc.sync.dma_start(
                    out=out_v[n], in_=u16[:].rearrange("p x y -> p (x y)")
                )
                continue

            # ---- sum(u^2) ----
            u2 = bf_pool.tile(ht_shape, fp16, name="u2")
            if sq_eng == 'act':
                nc.scalar.activation(u2[:], u16[:], AF.Square)
            elif sq_eng == 'gps':
                nc.gpsimd.tensor_tensor(u2[:], u16[:], u16[:], OP.mult)
            else:
                nc.vector.tensor_tensor(u2[:], u16[:], u16[:], OP.mult)
            ss = sm_pool.tile([CHUNK, E], fp32, name="ss")
            u2v = (
                u2[:].rearrange("p d e -> p e d") if epi == 'de' else u2[:]
            )
            nc.vector.tensor_reduce(
                ss[:], u2v, axis=mybir.AxisListType.X, op=OP.add
            )

            # ---- r = rsqrt(ss): bit-trick seed + Newton ----
            ti = sm_pool.tile([CHUNK, E], int32, name="ti")
            nc.vector.tensor_scalar(
                ti[:], ss[:].bitcast(int32), 1, None,
                op0=OP.logical_shift_right,
            )
            yi = sm_pool.tile([CHUNK, E], int32, name="yi")
            nc.vector.tensor_tensor(yi[:], magic[:], ti[:], OP.subtract)
            y = yi[:].bitcast(fp32)
            for _ in range(newton_iters):
                y2 = sm_pool.tile([CHUNK, E], fp32, name="y2")
                nc.vector.tensor_tensor(y2[:], y, y, OP.mult)
                tt = sm_pool.tile([CHUNK, E], fp32, name="tt")
                nc.vector.tensor_tensor(tt[:], ss[:], y2[:], OP.mult)
                ww = sm_pool.tile([CHUNK, E], fp32, name="ww")
                nc.vector.tensor_scalar(
                    ww[:], tt[:], -0.5, 1.5, op0=OP.mult, op1=OP.add
                )
                yn = sm_pool.tile([CHUNK, E], fp32, name="yn")
                nc.vector.tensor_tensor(yn[:], y, ww[:], OP.mult)
                y = yn[:]

            if ablate == 'norm':
                nc.sync.dma_start(
                    out=out_v[n], in_=u16[:].rearrange("p x y -> p (x y)")
                )
                continue

            # ---- scale and store fp16 ----
            o16 = bf_pool.tile(ht_shape, fp16, name="o16")
            if epi == 'de':
                y16 = sm_pool.tile([CHUNK, E], fp16, name="y16")
                nc.vector.tensor_copy(y16[:], y)
                yb = y16[:].unsqueeze(1).broadcast_to([CHUNK, D, E])
                nc.vector.tensor_tensor(o16[:], u16[:], yb, OP.mult)
            elif scale_eng == 'bcast':
                y16 = sm_pool.tile([CHUNK, E], fp16, name="y16")
                nc.vector.tensor_copy(y16[:], y)
                yb = y16[:].unsqueeze(2).broadcast_to([CHUNK, E, D])
                nc.vector.tensor_tensor(o16[:], u16[:], yb, OP.mult)
            else:
                if scale_eng == 'ts16':
                    y16 = sm_pool.tile([CHUNK, E], fp16, name="y16")
                    nc.vector.tensor_copy(y16[:], y)
                    ysrc = y16[:]
                else:
                    ysrc = y
                for e in range(E):
                    if scale_eng == 'act':
                        nc.scalar.mul(o16[:, e], u16[:, e], ysrc[:, e:e + 1])
                    else:
                        nc.vector.tensor_scalar(
                            o16[:, e], u16[:, e], ysrc[:, e:e + 1], None, op0=OP.mult
                        )
            nc.sync.dma_start(
                out=out_v[n], in_=o16[:].rearrange("p x y -> p (x y)")
            )

    nc.compile()
    return nc


def _get_nc():
    if "nc" not in _CACHE:
        _CACHE["nc"] = _build_nc()
    return _CACHE["nc"]


def make_in_maps(encoded_sents, prev_states, keys, U, V, W):
    enc = np.asarray(encoded_sents, dtype=np.float16)
    prev = np.asarray(prev_states, dtype=np.float16)
    kys = np.asarray(keys, dtype=np.float16)
    U = np.ascontiguousarray(np.asarray(U, dtype=np.float16))
    V = np.ascontiguousarray(np.asarray(V, dtype=np.float16))
    W = np.ascontiguousarray(np.asarray(W, dtype=np.float16))
    # per-chunk d-major transposes: [B_LOC, E, D] -> [N_CHUNKS, D, E, CHUNK]
    prevT = np.ascontiguousarray(
        prev.reshape(N_CORES, N_CHUNKS, CHUNK, E, D).transpose(0, 1, 4, 3, 2)
    )
    keysT = np.ascontiguousarray(
        kys.reshape(N_CORES, N_CHUNKS, CHUNK, E, D).transpose(0, 1, 4, 3, 2)
    )
    encT = np.ascontiguousarray(
        enc.reshape(N_CORES, N_CHUNKS, CHUNK, D).transpose(0, 1, 3, 2)
    )
    prev_row = np.ascontiguousarray(prev)
    # [B_LOC, E, D] -> [B_LOC, D, E] for the d-major epilogue layout
    prevR = np.ascontiguousarray(prev.transpose(0, 2, 1))
    in_maps = []
    for i in range(N_CORES):
        lo, hi = i * B_LOC, (i + 1) * B_LOC
        in_maps.append(
            {
                "prevT": prevT[i],
                "keysT": keysT[i],
                "encT": encT[i],
                "prev": prev_row[lo:hi],
                "prevR": prevR[lo:hi],
                "U": U,
                "V": V,
                "W": W,
            }
        )
    return in_maps


def kernel(encoded_sents, prev_states, keys, U, V, W):
    import sys

    if "/opt/trn_rl_repo" not in sys.path:
        sys.path.insert(0, "/opt/trn_rl_repo")
    from concourse.bass_utils import run_bass_kernel_spmd

    nc = _get_nc()
    in_maps = make_in_maps(encoded_sents, prev_states, keys, U, V, W)
    res = run_bass_kernel_spmd(nc, in_maps, list(range(N_CORES)))
    out = np.concatenate([res.results[i]["out"] for i in range(N_CORES)], axis=0)
    # default build uses the d-major epilogue: device output is [B, D, E]
    if out.shape[1] == D:
        out = out.transpose(0, 2, 1)
    return out.astype(np.float32)
